# revision 30
# baseline (speedup 1.0000x reference)
"""Multi-head attention TRN2 kernel (B=4, S=2048, E=128, H=8) on 8 NeuronCores.

Sharding: core c handles batch b = c // 2 and head group g = c % 2
(heads 4g .. 4g+3).  Each core computes the partial output
outT_partial[e_out, s] = sum_{h in group} (softmax(QK^T/sqrt(E)) V)_h @ Wo_h
for its batch, transposed.  Host sums the two head-group partials per batch,
transposes, and adds bo (plus the host-folded bv and bk contributions).

v3 design (vs v2 at ~164 us):
  - the exp stream (the ScalarE bottleneck at ~142 us) is split between
    ScalarE (native EXP activation) and the DVE via a custom 8-stage DVE op
    EXP16_ANT: attn = (kappa_t * ((1 + sc)^2 + 1))^16 ~= tau * e^(beta_t) *
    e^(16*sc), with sc = scores * SCALE / 16 (the /16 pre-folded into Wq on
    the host).  Softmax is scale-invariant so the global factor tau and the
    smooth ~0.1% approximation error wash out in the on-device
    normalization.
  - the Q bias never touches the device datapath: scores(Q+bq, K) =
    scores(Q, K) + beta[t] with beta = SCALE * q @ (Wk_h bq_h) computed on
    the host.  ScalarE chunks get it as the activation bias AP
    (beta + ln tau); DVE chunks get it folded into kappa_t =
    0.5 * (tau * e^beta)^(1/16).  The old per-chunk DVE bias adds become
    plain copies.
  - bk dropped (cancels in softmax); bv folded into the host-side output
    bias; attn weights fp8e4; V fp8; AV + denominator matmuls in DoubleRow.
"""

import sys

for _p in ("/opt/trn_rl_repo",):
    if _p not in sys.path:
        sys.path.insert(0, _p)

import numpy as np

import concourse.bass as bass
import concourse.mybir as mybir
import concourse.tile as tile
from concourse.bass_utils import run_bass_kernel_spmd

F32 = mybir.dt.float32
F16 = mybir.dt.float16
F8 = mybir.dt.float8e4
DR = mybir.MatmulPerfMode.DoubleRow
EXP = mybir.ActivationFunctionType.Exp

B, S, E, H = 4, 2048, 128, 8
NH = 4          # heads per core
TB = S // 128   # 16 t blocks
SW = 1024       # s-half width
NC = 512        # psum-bank chunk
SCALE = 1.0 / np.sqrt(E)
TAU = 4.0       # global attn-weight scale; cancels in normalization
LN_TAU = float(np.log(TAU))

# ---- custom DVE op: attn = (kappa_p * ((1 + x)^2 + 1))^16 -----------------
# 8 ALU stages (the v3 limit): ADD, SQ, MUL, ADD, SQ x4.  kappa is a [P,1]
# per-partition scalar AP riding the s0 slot (used twice).  Approximates
# (2*kappa)^16 * e^(16x) with |rel err| < ~0.1% over the score range.


def _exp16_reference(in0, in1, s0, s1, imm2):
    z = in0.astype(np.float32) + 1.0
    t = np.square(z) * s0 + s0
    for _ in range(4):
        t = np.square(t)
    return t.astype(np.float32)


def _mul_recip_reference(in0, in1, s0, s1, imm2):
    nx = (~in1.view(np.int32)).view(np.float32)
    y0 = nx * np.float32(s0)
    y1 = y0 * (np.float32(s1) - in1 * y0)
    return (in0.astype(np.float32) * y1).astype(np.float32)


def _register_op(name, spec):
    from concourse import dve_ops
    from concourse.dve_ops import DveOp
    from concourse.dve_spec import lower, _has_src1
    from concourse.dve_uop import DveOpSpec

    for op in dve_ops.OPS:
        if op.name == name:
            return op
    row = dve_ops._CUSTOM_DVE_ROW_BASE + len(dve_ops.OPS)
    sha = DveOpSpec(
        name=name, opcode=row, uops=lower(spec, ver="v3"),
        rd1_en=_has_src1(spec),
    ).sha("v3")
    op = DveOp(name, spec, subdim=False, uops_sha={"v3": sha})
    dve_ops.OPS.append(op)
    dve_ops._SUB_OPCODE_FOR_NAME[op.name] = row
    return op


def _make_ops():
    from concourse.dve_spec import Spec, Src0, Src1, C0, C1, One, sq, Bin, AluOp

    z = Src0 + One
    t = sq(z) * C0 + C0
    exp16 = _register_op(
        "EXP16_ANT", Spec(body=sq(sq(sq(sq(t)))), reference=_exp16_reference)
    )

    # out = in0 * recip1(in1): BITWISE_NOT exponent-flip seed + one inline
    # Newton-Raphson pass (~0.25% rel err — washes out in the softmax sum),
    # fusing the old reciprocal_approx_fast + tensor_mul pair.
    nx = Bin(AluOp.BITWISE_NOT, Src1, Src1)
    y0 = nx * C0
    y1 = y0 * (C1 - Src1 * y0)
    mulrecip = _register_op(
        "MUL_RECIP_ANT", Spec(body=Src0 * y1, reference=_mul_recip_reference)
    )
    return exp16, mulrecip


EXP16, MULRECIP = _make_ops()
_RC = {"s0": -0.23549792, "s1": 2.0017324}


def _full_scalar_tbs(h, sh):
    """t-blocks whose entire exp chunk runs on ScalarE (both halves), used
    to shed DVE load: the DVE half costs 0.75us vs ScalarE's 0.69us and
    the DVE also carries the normalization + evacuation glue."""
    if h == 0 and sh == 0:
        return (6, 12)
    if sh == 1:
        return (3, 8, 13)
    return (2, 6, 10, 14)

_prog_cache = {}


def build_program():
    if "nc" in _prog_cache:
        return _prog_cache["nc"]

    import concourse.bacc as bacc

    nc = bacc.Bacc("TRN2", target_bir_lowering=False, debug=False)

    qt_d = nc.dram_tensor("qT", [E, S], F16, kind="ExternalInput").ap()
    # packed weights: dim1 = (Wq, Wk, Wv, Wo); 4KB DRAM rows for DMA speed
    w_d = nc.dram_tensor("W", [E, 4, NH, E], F16, kind="ExternalInput").ap()
    # kappa / scalar-bias per (t_sub, h, tb):  [128, 2, NH*TB] f32
    kb_d = nc.dram_tensor("KB", [128, 2, NH * TB], F32, kind="ExternalInput").ap()
    out_d = nc.dram_tensor("out", [E, S], F16, kind="ExternalOutput").ap()

    with tile.TileContext(nc) as tc:
        _emit(nc, tc, qt_d, w_d, kb_d, out_d)

    nc.compile()
    _prog_cache["nc"] = nc
    return nc





def _emit(nc, tc, qt_d, w_d, kb_d, out_d):
    from collections import deque
    from contextlib import ExitStack

    ctx = ExitStack()
    consts = ctx.enter_context(tc.tile_pool(name="consts", bufs=1))
    heads = ctx.enter_context(tc.tile_pool(name="heads", bufs=2))
    attns = ctx.enter_context(tc.tile_pool(name="attns", bufs=2))
    works = ctx.enter_context(tc.tile_pool(name="works", bufs=2))
    psum_sc = ctx.enter_context(tc.tile_pool(name="psum_sc", bufs=4, space="PSUM"))
    psum_av = ctx.enter_context(tc.tile_pool(name="psum_av", bufs=2, space="PSUM"))
    psum_wk = ctx.enter_context(tc.tile_pool(name="psum_wk", bufs=2, space="PSUM"))

    # ---- constants / inputs ----
    # every engine owns a DMA issue queue: spread the input loads across all
    # five, ordered by first-use deadline, so the pre-compute head shrinks
    # from one-queue-serial (~6.4us) to ~max-single-transfer (~2.5us).
    qT = consts.tile([128, S], F16, tag="qT")  # [e, s]
    w_all4 = consts.tile([128, 4, NH, 128], F16, tag="w_all4")
    kb = consts.tile([128, 2, NH * TB], F32, tag="kb")
    nc.scalar.dma_start(out=w_all4[:, 0:1], in_=w_d[:, 0:1])  # Wq: prologue mm1
    nc.sync.dma_start(out=qT[:, 0:512], in_=qt_d[:, 0:512])
    nc.scalar.dma_start(out=kb, in_=kb_d)  # biases: first exp
    nc.sync.dma_start(out=w_all4[:, 1:2], in_=w_d[:, 1:2])  # Wk: prologue mm2
    nc.gpsimd.dma_start(out=w_all4[:, 2:3], in_=w_d[:, 2:3])  # Wv: v-blocks
    nc.sync.dma_start(out=qT[:, 512:1024], in_=qt_d[:, 512:1024])
    nc.scalar.dma_start(out=qT[:, 1024:2048], in_=qt_d[:, 1024:2048])
    nc.gpsimd.dma_start(out=w_all4[:, 3:4], in_=w_d[:, 3:4])  # Wo: 1st outproj
    wq = w_all4[:, 0]  # [e_in, h, e_out]  (host pre-scaled by SCALE/16)
    wk = w_all4[:, 1]
    wv = w_all4[:, 2]
    wo = w_all4[:, 3]  # [f, h, g]
    kap = kb[:, 0]  # [128, NH*TB]
    bia = kb[:, 1]

    ones8 = consts.tile([128, 2, 128], F8, tag="ones8")
    nc.vector.memset(ones8, 1.0)

    v8 = consts.tile([128, TB, NH * 128], F8, tag="v8")  # [t_sub, tb, (h f)]
    wv_all = wv.rearrange("e h f -> e (h f)")

    acc_a = [
        consts.tile([128, SW], F32, tag=f"acca{s}", name=f"acca{s}") for s in range(2)
    ]
    acc_b = [
        consts.tile([128, SW], F32, tag=f"accb{s}", name=f"accb{s}") for s in range(2)
    ]

    def v_block(tb, pool=None, tag="work"):
        pool = pool or psum_wk
        ps = pool.tile([128, NC], F32, tag=tag, name=f"vps_{tb}")
        nc.tensor.matmul(ps, lhsT=qT[:, tb * 128 : (tb + 1) * 128], rhs=wv_all,
                         start=True, stop=True)
        # the first segment carries all 16 of these on top of the exp
        # stream: split them across the two engines
        if tb % 2 == 1:
            nc.vector.tensor_copy(v8[:, tb, :], ps)
        else:
            nc.scalar.copy(v8[:, tb, :], ps)

    def proj_q(h, qt_h, j, eng="scalar"):
        ps = psum_wk.tile([128, NC], F32, tag="work", name=f"qp{h}_{j}")
        nc.tensor.matmul(ps, lhsT=wq[:, h, :], rhs=qT[:, j * 512 : (j + 1) * 512],
                         start=True, stop=True)
        if eng == "scalar":
            nc.scalar.copy(qt_h[:, j * 512 : (j + 1) * 512], ps)
        else:
            nc.vector.tensor_copy(qt_h[:, j * 512 : (j + 1) * 512], ps)

    def proj_k(h, kt_h, j, eng="vector"):
        ps = psum_wk.tile([128, NC], F32, tag="work", name=f"kp{h}_{j}")
        nc.tensor.matmul(ps, lhsT=wk[:, h, :], rhs=qT[:, j * 512 : (j + 1) * 512],
                         start=True, stop=True)
        if eng == "scalar":
            nc.scalar.copy(kt_h[:, j * 512 : (j + 1) * 512], ps)
        else:
            nc.vector.tensor_copy(kt_h[:, j * 512 : (j + 1) * 512], ps)

    def alloc_head(h):
        qt_h = heads.tile([128, S], F16, tag="QT", name=f"qt{h}")  # [f, s]
        kt_h = heads.tile([128, S], F16, tag="KT", name=f"kt{h}")  # [f, t]
        return qt_h, kt_h

    # prologue: only what the first scores matmul needs; the Q evacuations
    # are pure copies now (bias lives in the exp), done on the idle ScalarE
    cur = alloc_head(0)
    ps = psum_wk.tile([128, NC], F32, tag="work", name="qp0_0p")
    nc.tensor.matmul(ps, lhsT=wq[:, 0, :], rhs=qT[:, 0:512], start=True, stop=True)
    nc.scalar.copy(cur[0][:, 0:512], ps)
    ps = psum_wk.tile([128, NC], F32, tag="work", name="kp0_0p")
    nc.tensor.matmul(ps, lhsT=wk[:, 0, :], rhs=qT[:, 0:512], start=True, stop=True)
    nc.vector.tensor_copy(cur[1][:, 0:512], ps)  # DVE, parallel to ScalarE
    ps = psum_wk.tile([128, NC], F32, tag="work", name="qp0_1p")
    nc.tensor.matmul(ps, lhsT=wq[:, 0, :], rhs=qT[:, 512:1024], start=True, stop=True)
    nc.scalar.copy(cur[0][:, 512:1024], ps)

    osb_tiles = {}

    def out_proj(h, sh, c, ztn):
        # output projection + head accumulation for chunk c of segment (h, sh)
        wo_ps = psum_wk.tile([128, NC], F32, tag="work", name=f"wop{h}{sh}{c}")
        nc.tensor.matmul(wo_ps, lhsT=wo[:, h, :], rhs=ztn, start=True, stop=True)
        asl = slice(c * 512, (c + 1) * 512)
        if h == 0:
            nc.vector.tensor_copy(acc_a[sh][:, asl], wo_ps)
        elif h == 1:
            nc.vector.tensor_add(acc_b[sh][:, asl], acc_a[sh][:, asl], wo_ps)
        elif h == 2:
            nc.vector.tensor_add(acc_a[sh][:, asl], acc_b[sh][:, asl], wo_ps)
        else:
            if sh not in osb_tiles:
                osb_tiles[sh] = works.tile([128, SW], F16, tag="osb",
                                           name=f"osb{sh}")
            osb = osb_tiles[sh]
            if sh == 1 and c == 1:
                # very last output chunk: split the add+DMA in half so the
                # final transfer starts as early as possible
                for q in range(2):
                    qsl = slice(c * 512 + q * 256, c * 512 + (q + 1) * 256)
                    nc.vector.tensor_add(
                        osb[:, qsl], acc_a[sh][:, qsl],
                        wo_ps[:, q * 256 : (q + 1) * 256],
                    )
                    ssl = slice(sh * SW + c * 512 + q * 256,
                                sh * SW + c * 512 + (q + 1) * 256)
                    nc.sync.dma_start(out=out_d[:, ssl], in_=osb[:, qsl])
            else:
                nc.vector.tensor_add(osb[:, asl], acc_a[sh][:, asl], wo_ps)
                # per-chunk DMA: one half leaves while the other is added
                ssl = slice(sh * SW + c * 512, sh * SW + (c + 1) * 512)
                nc.sync.dma_start(out=out_d[:, ssl], in_=osb[:, asl])

    nxt = None
    deferred = None  # tail work from the previous segment
    for h in range(NH):
        qt_h, kt_h = cur
        for sh in range(2):
            # per-iteration emission schedule. pre[tb] runs at the top of
            # iteration tb (projection drip / deferred out-proj: deps are
            # already met, so they never head-block the FIFO); post[tb] runs
            # after scores+exp of iteration tb (AV/dns pairs whose exp dep
            # clears exactly when the PE FIFO reaches them).
            pre = [[] for _ in range(TB + 1)]
            post = [[] for _ in range(TB + 1)]

            def place(tb, thunk):
                pre[min(tb, TB)].append(thunk)

            def place_post(tb, thunk):
                post[min(tb, TB)].append(thunk)

            # previous segment's tail, then its normalization, then its
            # out-projection, then this segment's projection drip, then the
            # denominator accumulation — strictly in that order so the two
            # rotating work banks never force a FIFO head-block.
            base = 0
            if deferred is not None:
                n_mms = len(deferred["mms"])
                for i, t in enumerate(deferred["mms"]):
                    place_post(1 + i // 2, t)
                base = 1 + (n_mms + 1) // 2
                place_post(base, deferred["norm"][0])
                place_post(base + 1, deferred["norm"][1])
                place_post(base + 2, deferred["wo"][0])
                place_post(base + 3, deferred["wo"][1])
                deferred = None

            drip = deque()
            drip0 = 0
            if h == 0 and sh == 0:
                drip0 = 1  # the V-block weights (Wv) land a moment later
                vb = lambda t: (lambda tt=t: v_block(tt))
                pk = lambda j: (lambda jj=j: proj_k(0, kt_h, jj))
                pq = lambda j: (lambda jj=j: proj_q(0, qt_h, jj))
                # V0/V1 borrow the (idle until slot 3) zts banks so they
                # don't wait on the prologue's work-bank evacuations
                vz = lambda t: (lambda tt=t: v_block(tt, psum_av, "zt"))
                drip += [vz(0), vz(1), pk(1), vb(2), vb(3), pk(2), vb(4), vb(5),
                         pq(2), vb(6), vb(7), pk(3), vb(8), vb(9), pq(3)]
                drip += [vb(t) for t in range(10, TB)]
            if sh == 1 and h + 1 < NH:
                nxt = alloc_head(h + 1)
                hh, nq, nk = h + 1, nxt[0], nxt[1]
                # Q evacuations ride ScalarE (it has slack); K stays DVE
                drip += [lambda j=j: proj_q(hh, nq, j, "scalar") for j in range(4)]
                drip += [lambda j=j: proj_k(hh, nk, j, "vector") for j in range(4)]
                drip0 = 4
            n_drip = len(drip)
            for i, t in enumerate(drip):
                place_post(drip0 + i // 2, t)
            first_free = max(drip0 + (n_drip + 1) // 2, 5, base + 4)

            s0 = sh * SW
            # two attn tiles: ScalarE owns columns 0:512 (attnS), the DVE
            # owns 512:1024 (attnD).  Separate tiles keep the overlap
            # tracker from serializing the two engines' writes.
            attnS = attns.tile([128, TB, NC], F8, tag="attnS", name=f"aS{h}{sh}")
            attnD = attns.tile([128, TB, NC], F8, tag="attnD", name=f"aD{h}{sh}")
            at_c = (attnS, attnD)
            zts = [
                psum_av.tile([128, NC], F32, tag="zt", name=f"zt{h}{sh}{c}")
                for c in range(2)
            ]
            dn_tiles = {}

            def dns_pair(p, c, at=at_c, dn=dn_tiles, hh=h, ss=sh):
                if c not in dn:
                    dn[c] = psum_wk.tile(
                        [128, NC], F32, tag="work", name=f"dn{hh}{ss}{c}"
                    )
                nc.tensor.matmul(
                    dn[c],
                    lhsT=ones8,
                    rhs=at[c][:, 2 * p : 2 * p + 2, :],
                    start=(p == 0), stop=(p == 7), perf_mode=DR,
                )

            def av_pair(p, c, at=at_c, z=zts, hh=h):
                nc.tensor.matmul(
                    z[c],
                    lhsT=v8[:, 2 * p : 2 * p + 2, hh * 128 : (hh + 1) * 128],
                    rhs=at[c][:, 2 * p : 2 * p + 2, :],
                    start=(p == 0), stop=(p == 7), perf_mode=DR,
                )

            last_seg = h == NH - 1 and sh == 1
            for p in range(8):
                if last_seg and p == 7:
                    # final drain: chunk-0 matmuls only; chunk 1 is emitted
                    # in the tail interleaved with chunk-0's normalization
                    place_post(16, lambda: (dns_pair(7, 0), av_pair(7, 0)))
                    continue
                place_post(
                    max(2 * p + 3, first_free + p),
                    lambda p=p, f=dns_pair: (f(p, 0), f(p, 1)),
                )
                place_post(
                    2 * p + 3, lambda p=p, f=av_pair: (f(p, 0), f(p, 1))
                )

            for tb in range(TB):
                for t in pre[tb]:
                    t()
                # one PSUM tile per (tb, engine-half): each exp chain
                # releases its own buffer independently — the pool-rotation
                # WAR never couples the two engines
                sc0 = psum_sc.tile([128, NC], F32, tag="sc", name=f"sc{h}{sh}{tb}a")
                nc.tensor.matmul(
                    sc0,
                    lhsT=kt_h[:, tb * 128 : (tb + 1) * 128],
                    rhs=qt_h[:, s0 : s0 + 512],
                    start=True, stop=True,
                )
                sc1 = psum_sc.tile([128, NC], F32, tag="sc", name=f"sc{h}{sh}{tb}b")
                nc.tensor.matmul(
                    sc1,
                    lhsT=kt_h[:, tb * 128 : (tb + 1) * 128],
                    rhs=qt_h[:, s0 + 512 : s0 + 1024],
                    start=True, stop=True,
                )
                hi = h * TB + tb
                nc.scalar.activation(
                    attnS[:, tb, :], sc0, EXP, scale=16.0,
                    bias=bia[:, hi : hi + 1],
                )
                if tb in _full_scalar_tbs(h, sh):
                    nc.scalar.activation(
                        attnD[:, tb, :], sc1, EXP, scale=16.0,
                        bias=bia[:, hi : hi + 1],
                    )
                else:
                    nc.vector._custom_dve(
                        EXP16, out=attnD[:, tb, :], in0=sc1,
                        s0=kap[:, hi : hi + 1],
                    )
                for t in post[tb]:
                    t()

            holder = {}

            def norm_chunk(c, dn=dn_tiles, z=zts, hol=holder, hh=h, ss=sh):
                recip = works.tile([128, NC], F32, tag="recip",
                                   name=f"rc{hh}{ss}{c}")
                nc.vector.reciprocal_approx_fast(recip, dn[c])
                ztn = works.tile([128, NC], F16, tag="ztn", name=f"zn{hh}{ss}{c}")
                # fused multiply-by-reciprocal is impossible here: the DVE
                # reads at most one non-scalar PSUM operand and both zts and
                # dn live in PSUM
                nc.vector.tensor_mul(ztn, z[c], recip)
                hol[c] = ztn

            if last_seg:
                # final segment: drain immediately; chunk-0's normalization
                # overlaps chunk-1's last matmuls
                for t in pre[TB]:
                    t()
                for t in post[TB]:
                    t()
                norm_chunk(0)
                dns_pair(7, 1)
                av_pair(7, 1)
                out_proj(h, sh, 0, holder[0])
                norm_chunk(1)
                out_proj(h, sh, 1, holder[1])
            else:
                deferred = {
                    "mms": pre[TB] + post[TB],
                    "norm": [lambda c=c, f=norm_chunk: f(c) for c in range(2)],
                    "wo": [
                        lambda c=c, hol=holder, hh=h, ss=sh: out_proj(
                            hh, ss, c, hol[c]
                        )
                        for c in range(2)
                    ],
                }
        if h + 1 < NH:
            cur = nxt

    ctx.close()


def _in_maps(inputs):
    q = np.asarray(inputs["q"], dtype=np.float32)
    Wq = np.asarray(inputs["Wq"], dtype=np.float32)
    bq = np.asarray(inputs["bq"], dtype=np.float32)
    Wk = np.asarray(inputs["Wk"], dtype=np.float32)
    Wv = np.asarray(inputs["Wv"], dtype=np.float32)
    Wo = np.asarray(inputs["Wo"], dtype=np.float32).reshape(H, E, E)

    Wq_s = Wq * (SCALE / 16.0)  # scores arrive as x/16 on device

    def warr(w, hs):  # [h, e_in, e_out] slice -> [e_in, h, e_out] f16
        return w[hs].transpose(1, 0, 2).astype(np.float16)

    maps = []
    for c in range(8):
        b = c // 2
        hs = slice(4 * (c % 2), 4 * (c % 2) + 4)
        w_all = np.ascontiguousarray(
            np.stack([warr(Wq_s, hs), warr(Wk, hs), warr(Wv, hs), warr(Wo, hs)], 1)
        )  # [e_in, 4, h, e_out]

        # beta_h[t] = SCALE * (q[b] @ (Wk_h @ bq_h))[t]  — the Q-bias term
        gamma = np.einsum("hef,hf->he", Wk[hs], bq[hs])  # [4, E]
        beta = SCALE * (q[b] @ gamma.T)  # [S, 4]
        beta_l = beta.reshape(TB, 128, NH).transpose(1, 2, 0)  # [t_sub, h, tb]
        beta_l = np.ascontiguousarray(beta_l).reshape(128, NH * TB)
        kap = (0.5 * TAU ** (1.0 / 16.0) * np.exp(beta_l / 16.0)).astype(
            np.float32
        )
        bia = (beta_l + LN_TAU).astype(np.float32)
        kb = np.ascontiguousarray(np.stack([kap, bia], axis=1))  # [128,2,64]

        maps.append(
            {
                "qT": np.ascontiguousarray(q[b].T).astype(np.float16),
                "W": w_all,
                "KB": kb,
            }
        )
    return maps


def kernel(**inputs):
    nc = build_program()
    maps = _in_maps(inputs)
    res = run_bass_kernel_spmd(nc, maps, core_ids=list(range(8)))
    bo = np.asarray(inputs["bo"], dtype=np.float32)
    bv = np.asarray(inputs["bv"], dtype=np.float32)
    Wo = np.asarray(inputs["Wo"], dtype=np.float32).reshape(H, E, E)
    # V-bias contribution folded out of the device kernel:
    # sum_h softmax(..)@ (qWv + bv) @ Wo_h = device_partials + sum_h bv_h @ Wo_h
    bo_eff = bo + np.einsum("he,hef->f", bv, Wo).astype(np.float32)
    out = np.empty((B, S, E), dtype=np.float32)
    for b in range(B):
        part = res.results[2 * b]["out"].astype(np.float32) + res.results[
            2 * b + 1
        ]["out"].astype(np.float32)
        out[b] = part.T + bo_eff
    return out
